# revision 16
# baseline (speedup 1.0000x reference)
"""Trainium2 Bass kernel for nn_LowRankDynamicConv.

Math (per sample b):
  combined = [phrase_slot[b] | eos]                       [N, 2C]
  h        = relu(combined @ W1 + b1)                     [N, 4C]
  proj     = (h @ W2 + b2) viewed as [N*C, R]             [4096, 32]
  y        = x[b] @ proj   with x[b] = context_emb[b] as  [T, N*C]
  out_k[t] = sum_j y[t + j - pad_k] @ kparam_k[:, :, j]   [T, C] for k in (1,3,5)
  out      = relu(LN(concat(out_k) @ Wo + bo))            [T, C]

This is the low-rank refactor of the reference's dense dynamic conv:
  out_k = sum_j shift_j(x) @ (proj @ kparam_k[:,:,j]) == sum_j shift_j(x @ proj) @ kparam_k

Sharding: data-parallel over batch, 2 samples per core. W2 (the one big
weight, 32MB) is column-sharded: every core computes the proj slice for
ALL 16 samples with its 1/8 of W2's columns, then an on-chip AllToAll
redistributes so each core holds the full proj for its own 2 samples.
This cuts per-core HBM traffic by ~28MB vs replicating W2.

Precision: weights and activations feeding matmuls are cast to bf16
(fast weight load + 1 cyc/row); accumulation stays f32 in PSUM, and the
LN epilogue is f32. x is PE-transposed in f32 and stored bf16. Measured
end-to-end rel err vs the f32 reference is well under the 2e-2 gate.
"""
import sys

sys.path.insert(0, "/opt/trn_rl_repo")

import numpy as np

import concourse.bass as bass
import concourse.mybir as mybir
import concourse.tile as tile
from concourse import bacc
from concourse.bass_utils import run_bass_kernel_spmd
from concourse.masks import make_identity

F32 = mybir.dt.float32
BF16 = mybir.dt.bfloat16
RELU = mybir.ActivationFunctionType.Relu
SQRT = mybir.ActivationFunctionType.Sqrt

NCORES = 8
BPC = 2                    # samples per core
T, N, C, R = 1024, 16, 256, 32
BN = NCORES * BPC * N      # 256 (b, n) rows across ALL samples
NCF = N * C                # 4096 flattened (n, c) contraction dim
CH = NCF // 128            # 32 nc-chunks of 128
TQ = 2                     # t processed in 2 chunks of 512
TCHUNK = T // TQ           # 512
PAD = 2                    # max conv pad (k=5)
YW = T + 2 * PAD           # padded y width, 1028
W2C = C * R // NCORES      # 1024 W2 columns per core
# (kernel_size, j) pairs in feat-concat order: k1 | k3 | k5
KJ = [(1, [0]), (3, [0, 1, 2]), (5, [0, 1, 2, 3, 4])]
NJ = 9                     # total j count


def _broadcast_ap(ap, parts):
    """DMA access pattern replicating a 1D/2D DRAM tensor across `parts` partitions."""
    a = ap
    return bass.AP(tensor=a.tensor, offset=a.offset, ap=[[0, parts]] + list(a.ap))


def _build():
    nc = bacc.Bacc("TRN2", num_devices=NCORES)

    xb = nc.dram_tensor("xb", [BPC, T, NCF], F32, kind="ExternalInput")
    phrase = nc.dram_tensor("phrase", [BN, C], F32, kind="ExternalInput")
    eos = nc.dram_tensor("eos", [C], F32, kind="ExternalInput")
    w1 = nc.dram_tensor("w1", [2 * C, 4 * C], F32, kind="ExternalInput")
    b1 = nc.dram_tensor("b1", [4 * C], F32, kind="ExternalInput")
    w2s = nc.dram_tensor("w2s", [4 * C, W2C], F32, kind="ExternalInput")
    b2s = nc.dram_tensor("b2s", [W2C], F32, kind="ExternalInput")
    kjoin = nc.dram_tensor("kjoin", [NJ, R, C], F32, kind="ExternalInput")
    wo = nc.dram_tensor("wo", [3 * C, C], F32, kind="ExternalInput")
    bo = nc.dram_tensor("bo", [C], F32, kind="ExternalInput")
    gamma = nc.dram_tensor("gamma", [C], F32, kind="ExternalInput")
    beta = nc.dram_tensor("beta", [C], F32, kind="ExternalInput")
    out = nc.dram_tensor("out", [BPC, T, C], F32, kind="ExternalOutput")

    with tile.TileContext(nc) as tc:
        with tc.tile_pool(name="keep", bufs=1) as keep, \
             tc.tile_pool(name="dram", bufs=1, space="DRAM") as dram, \
             tc.tile_pool(name="tp", bufs=2, space="PSUM") as tp, \
             tc.tile_pool(name="pXn", bufs=2) as pXn, \
             tc.tile_pool(name="pX", bufs=2) as pX:
            ident = keep.tile([128, 128], F32)
            make_identity(nc, ident)
            ones1 = keep.tile([1, 128], BF16)
            nc.vector.memset(ones1, 1.0)
            eps = keep.tile([128, 1], F32)
            nc.vector.memset(eps, 1e-5)

            # LN params + output bias, broadcast across partitions
            gsb = keep.tile([128, C], F32)
            nc.sync.dma_start(gsb, _broadcast_ap(gamma[:], 128))
            bsb = keep.tile([128, C], F32)
            nc.sync.dma_start(bsb, _broadcast_ap(beta[:], 128))
            bosb = keep.tile([128, C], F32)
            nc.sync.dma_start(bosb, _broadcast_ap(bo[:], 128))

            # bf16 stage-4/5 weights (loaded in phase A via the shared scratch tag)
            wob = keep.tile([128, 6, C], BF16)
            kj1b = keep.tile([R, C], BF16)        # k1 tap
            kjS3b = keep.tile([3 * R, C], BF16)   # k3 taps stacked on (j, r)
            kjS5b = keep.tile([4 * R, C], BF16)   # k5 taps j=0..3 stacked
            kj5bb = keep.tile([R, C], BF16)       # k5 tap j=4

            # y^T buffers, one per sample, shift-replicated: partition block s
            # (s=0..3) holds Y[32s+r, c] = y[r, c - PAD + s] so stacked conv
            # taps read one contiguous [*, TCHUNK] slice per kernel size.
            ysb = []
            for b in range(BPC):
                y = keep.tile([128, YW], BF16, name=f"ysb{b}")
                for s in range(4):
                    if PAD - s > 0:
                        nc.vector.memset(y[32 * s:32 * (s + 1), 0:PAD - s], 0.0)
                    nc.vector.memset(y[32 * s:32 * (s + 1), PAD + T - s:YW], 0.0)
                ysb.append(y)

            # proj for this core's samples: [nc%128 part, (b, ch), r], bf16
            projw = keep.tile([128, BPC * CH, R], BF16)

            # AllToAll bounce buffers: [src/dst core, bn rows, W2-col slice]
            in_b = dram.tile([NCORES, BPC * N, W2C], BF16, tag="inb")
            out_b = dram.tile([NCORES, BPC * N, W2C], BF16, tag="outb")

            # ---- phase A: proj slices for all samples, AllToAll, reshard ----
            with tc.tile_pool(name="pA", bufs=1) as pA, \
                 tc.tile_pool(name="pAs", bufs=2) as pAs, \
                 tc.tile_pool(name="psA", bufs=2, space="PSUM") as psA, \
                 tc.high_priority():
                # combined^T [c2%128 part, ko, bn] for ALL bn rows, bf16
                phsb = pA.tile([128, 2, C], F32)
                nc.sync.dma_start(phsb, phrase[:, :].rearrange("(rt p) c -> p rt c", p=128))
                eossb = pA.tile([128, 2], F32)
                nc.sync.dma_start(eossb, eos[:].rearrange("(o p) -> p o", p=128))
                combT = pA.tile([128, 4, BN], BF16)
                for rt in range(2):
                    for ko in range(2):
                        pht = psA.tile([128, 128], F32, tag="h")
                        nc.tensor.transpose(pht, phsb[:, rt, ko * 128:(ko + 1) * 128],
                                            ident)
                        nc.vector.tensor_copy(combT[:, ko, rt * 128:(rt + 1) * 128], pht)
                for o in range(2):
                    nc.vector.tensor_copy(
                        combT[:, 2 + o, :],
                        eossb[:, o:o + 1].to_broadcast((128, BN)))

                # W1 -> bf16 [c2%128, ko, m]; b1 -> [m%128, mo]
                w1b = pA.tile([128, 4, 4 * C], BF16)
                for hw in range(2):
                    w1q = keep.tile([128, 2, 4 * C], F32, tag="stg", name=f"w1q{hw}")
                    nc.sync.dma_start(
                        w1q, w1[hw * 256:(hw + 1) * 256, :]
                        .rearrange("(ko p) m -> p ko m", p=128))
                    nc.vector.tensor_copy(w1b[:, 2 * hw:2 * hw + 2, :], w1q)
                b1sb = pA.tile([128, 8], F32)
                nc.sync.dma_start(b1sb, b1[:].rearrange("(mo p) -> p mo", p=128))

                # h^T [m%128 part, mo, bn] = relu(W1^T combined + b1), bf16
                hT = pA.tile([128, 8, BN], BF16)
                for mo in range(8):
                    phm = psA.tile([128, BN], F32, tag="h")
                    for ko in range(4):
                        nc.tensor.matmul(phm, w1b[:, ko, mo * 128:(mo + 1) * 128],
                                         combT[:, ko, :],
                                         start=(ko == 0), stop=(ko == 3))
                    nc.scalar.activation(out=hT[:, mo, :], in_=phm, func=RELU,
                                         bias=b1sb[:, mo:mo + 1], scale=1.0)

                # b2 -> bf16 (staged early; the rank-1 bias add closes each group)
                b2f = keep.tile([1, W2C], F32, tag="stg", name="b2f")
                nc.sync.dma_start(b2f, b2s[:].rearrange("(p q) -> p q", p=1))
                b2b = pA.tile([1, W2C], BF16)
                nc.vector.tensor_copy(b2b, b2f)

                # W2 slice -> bf16 in quarters; pp accumulates as quarters land
                w2b = pA.tile([128, 8, W2C], BF16)
                groups = [(0, 0), (0, 1), (1, 0), (1, 1)]
                pps = [psA.tile([128, 512], F32, tag="pj", bufs=4, name=f"pp{g}")
                       for g in range(4)]
                for qw in range(4):
                    w2q = pA.tile([128, 2, W2C], F32, tag="w2q", bufs=2, name=f"w2q{qw}")
                    nc.sync.dma_start(
                        w2q, w2s[qw * 256:(qw + 1) * 256, :]
                        .rearrange("(ko p) q -> p ko q", p=128))
                    nc.vector.tensor_copy(w2b[:, 2 * qw:2 * qw + 2, :], w2q)
                    for g, (rt, cc) in enumerate(groups):
                        for k2 in range(2):
                            ko = 2 * qw + k2
                            nc.tensor.matmul(
                                pps[g], hT[:, ko, rt * 128:(rt + 1) * 128],
                                w2b[:, ko, cc * 512:(cc + 1) * 512],
                                start=(ko == 0), stop=False)
                for g, (rt, cc) in enumerate(groups):
                    nc.tensor.matmul(pps[g], ones1[:, 0:128],
                                     b2b[:, cc * 512:(cc + 1) * 512],
                                     start=False, stop=True)
                    ppsb = pAs.tile([128, 512], BF16, tag="ppsb")
                    nc.scalar.copy(ppsb, pps[g])
                    nc.scalar.dma_start(
                        in_b[rt * 4:(rt + 1) * 4, :, cc * 512:(cc + 1) * 512], ppsb)
                wof = keep.tile([128, 6, C], F32, tag="stg", name="wof")
                nc.sync.dma_start(wof, wo[:, :].rearrange("(fc p) co -> p fc co", p=128))
                nc.vector.tensor_copy(wob, wof)
                for nm, dst, lo, hi in (("kj1f", kj1b, 0, 1), ("kjS3f", kjS3b, 1, 4),
                                        ("kjS5f", kjS5b, 4, 8), ("kj5bf", kj5bb, 8, 9)):
                    kjf = keep.tile([(hi - lo) * R, C], F32, tag="stg", name=nm)
                    nc.sync.dma_start(
                        kjf, kjoin[lo:hi, :, :].rearrange("j r d -> (j r) d"))
                    nc.vector.tensor_copy(dst, kjf)

                nc.gpsimd.collective_compute(
                    "AllToAll",
                    mybir.AluOpType.bypass,
                    replica_groups=[list(range(NCORES))],
                    ins=[in_b[:, :, :].opt()],
                    outs=[out_b[:, :, :].opt()],
                )

                # reshard: out_b[i=qh, b*16+n, (2*ql+e)*32+r] -> projw[qh*16+ql, (b,n,e), r]
                # i.e. partition q holds c = 2q+e; pairs match the bf16-pair transposes.
                ob = out_b[:, :, :]
                for qh in range(NCORES):
                    src = bass.AP(
                        tensor=ob.tensor,
                        offset=ob.offset + qh * (BPC * N * W2C),
                        ap=[[64, 16],            # ql -> partition within the qh-group
                            [W2C, BPC * N],      # (b, n) merged
                            [1, 2 * R]])         # (e, r) merged
                    nc.scalar.dma_start(projw[qh * 16:(qh + 1) * 16, :, :], src)

            # ---- phase X: per (sample, t-chunk) stream ------------------------
            with tc.tile_pool(name="yp", bufs=2, space="PSUM") as yp, \
                 tc.tile_pool(name="fp", bufs=2, space="PSUM") as fp, \
                 tc.tile_pool(name="op", bufs=2, space="PSUM") as op, \
                 tc.tile_pool(name="pX2", bufs=1) as pX2, \
                 tc.tile_pool(name="pXs", bufs=2) as pXs:
                def emit_feat(b, tq):
                    t0 = tq * TCHUNK
                    # stage 4: feat^T[d-block, t]; conv taps stacked on (j, r)
                    # partitions so each kernel size is 1-2 matmuls per d-block
                    featT = pX.tile([128, 6, TCHUNK], BF16, tag="featT", bufs=1)
                    yb = ysb[b]
                    for kb in range(3):
                        for dc in range(2):
                            dsl = slice(dc * 128, (dc + 1) * 128)
                            pf = fp.tile([128, TCHUNK], F32, tag="f")
                            if kb == 0:
                                nc.tensor.matmul(
                                    pf, kj1b[:, dsl],
                                    yb[0:32, PAD + t0:PAD + t0 + TCHUNK],
                                    start=True, stop=True)
                            elif kb == 1:
                                nc.tensor.matmul(
                                    pf, kjS3b[:, dsl],
                                    yb[0:96, PAD + t0 - 1:PAD + t0 - 1 + TCHUNK],
                                    start=True, stop=True)
                            else:
                                nc.tensor.matmul(
                                    pf, kjS5b[:, dsl],
                                    yb[0:128, PAD + t0 - 2:PAD + t0 - 2 + TCHUNK],
                                    start=True, stop=False)
                                nc.tensor.matmul(
                                    pf, kj5bb[:, dsl],
                                    yb[0:32, PAD + t0 + 2:PAD + t0 + 2 + TCHUNK],
                                    start=False, stop=True)
                            nc.vector.tensor_copy(featT[:, kb * 2 + dc, :], pf)

                    # stage 5: out[t%128, co] = feat @ Wo, then LN + relu
                    for ts in range(TCHUNK // 128):
                        po = op.tile([128, C], F32, tag="o")
                        for fc in range(6):
                            nc.tensor.matmul(
                                po, featT[:, fc, ts * 128:(ts + 1) * 128],
                                wob[:, fc, :], start=(fc == 0), stop=(fc == 5))
                        osb = pXs.tile([128, C], F32, tag="osb")
                        nc.vector.tensor_add(osb, po, bosb)
                        st = pXs.tile([128, 6], F32, tag="st")
                        nc.vector.bn_stats(out=st, in_=osb)
                        mv = pXs.tile([128, 2], F32, tag="mv")
                        nc.vector.bn_aggr(out=mv, in_=st)
                        # rstd = 1/sqrt(var + eps)
                        rs = pXs.tile([128, 1], F32, tag="rs")
                        nc.scalar.activation(out=rs, in_=mv[:, 1:2], func=SQRT,
                                             bias=eps, scale=1.0)
                        nc.vector.reciprocal(rs, rs)
                        nc.vector.tensor_scalar(osb, osb, mv[:, 0:1], rs,
                                                mybir.AluOpType.subtract,
                                                mybir.AluOpType.mult)
                        nc.vector.tensor_mul(osb, osb, gsb)
                        nc.vector.tensor_add(osb, osb, bsb)
                        nc.vector.tensor_scalar_max(osb, osb, 0.0)
                        nc.sync.dma_start(
                            out[b, t0 + ts * 128:t0 + (ts + 1) * 128, :], osb)

                for b in range(BPC):
                    for tq in range(TQ):
                        chunk = b * TQ + tq
                        t0 = tq * TCHUNK
                        # x [t%128, nc] f32 -> bf16, then PE-transpose bf16 PAIRS
                        # viewed as f32 (bit-exact): xT2[q, n, t] = packed
                        # (x[t, n*256+2q], x[t, n*256+2q+1])
                        xpool = pX if chunk < 2 else pX2
                        xT2 = xpool.tile([128, N, TCHUNK], F32, tag="xT")
                        for ts in range(TCHUNK // 128):
                            xn = pXn.tile([128, NCF], F32, tag="xn")
                            nc.sync.dma_start(
                                xn, xb[b, t0 + ts * 128:t0 + (ts + 1) * 128, :])
                            xc = pXn.tile([128, NCF], BF16, tag="xc")
                            # early chunks: cast on ACT (DVE feeds phase A's
                            # w2->bf16); later chunks: cast on DVE
                            if chunk < 2:
                                nc.scalar.copy(xc, xn)
                            else:
                                nc.vector.tensor_copy(xc, xn)
                            xc32 = xc[:, :].bitcast(F32)
                            for pg in range(4):   # 4 pair-block transposes per bank
                                pt = tp.tile([128, 4, 128], F32, tag="tp")
                                for q in range(4):
                                    blk = pg * 4 + q
                                    nc.tensor.transpose(
                                        pt[:, q, :],
                                        xc32[:, blk * 128:(blk + 1) * 128], ident)
                                dst = xT2[:, pg * 4:(pg + 1) * 4,
                                          ts * 128:(ts + 1) * 128]
                                if chunk < 2 or pg % 2 == 0:
                                    nc.vector.tensor_copy(dst, pt)
                                else:
                                    nc.scalar.copy(dst, pt)

                        # stage 3: y^T[r, t-chunk] = sum_(n,e) proj^T @ xT2
                        # (even/odd bf16 lanes of each packed pair via stride-2 APs)
                        xv = xT2[:, :, :].bitcast(BF16)  # [128, N, 2*TCHUNK]
                        py = yp.tile([R, TCHUNK], F32, tag="y")
                        for pc in range(N):
                            for e in range(2):
                                rhs = xv[:, pc, :].rearrange(
                                    "p (t e) -> p e t", e=2)[:, e, :]
                                nc.tensor.matmul(
                                    py, projw[:, (b * N + pc) * 2 + e, :], rhs,
                                    start=(pc == 0 and e == 0),
                                    stop=(pc == N - 1 and e == 1))
                        for s in range(4):
                            start = PAD + t0 - s
                            soff = max(0, -start)
                            nc.vector.tensor_copy(
                                ysb[b][32 * s:32 * (s + 1),
                                       start + soff:start + TCHUNK],
                                py[:, soff:TCHUNK])
                        # stage 4/5 lag one chunk: feat(tq-1) needs y[tq]'s
                        # first PAD columns (k=5 right overhang)
                        if tq > 0:
                            emit_feat(b, tq - 1)
                    emit_feat(b, TQ - 1)

    nc.compile()
    return nc


_NC = None


def _get_nc():
    global _NC
    if _NC is None:
        _NC = _build()
    return _NC


def _shard(inputs):
    """Split full inputs into per-core input maps (pure slicing/stacking)."""
    x = np.ascontiguousarray(inputs["context_emb"], dtype=np.float32)
    B = x.shape[0]
    assert B == NCORES * BPC
    x = x.reshape(B, T, NCF)
    ph = np.ascontiguousarray(
        inputs["phrase_slot"], dtype=np.float32).reshape(BN, C)
    w2 = np.asarray(inputs["W2"], dtype=np.float32)
    b2 = np.asarray(inputs["b2"], dtype=np.float32)
    kjoin = np.ascontiguousarray(np.concatenate(
        [np.moveaxis(inputs[f"k{k}"], 2, 0) for k in (1, 3, 5)], axis=0),
        dtype=np.float32)  # [9, 32, 256]
    shared = {
        "phrase": ph,
        "eos": np.ascontiguousarray(inputs["eos_slot"].reshape(C), dtype=np.float32),
        "w1": np.ascontiguousarray(inputs["W1"], dtype=np.float32),
        "b1": np.ascontiguousarray(inputs["b1"], dtype=np.float32),
        "kjoin": kjoin,
        "wo": np.ascontiguousarray(inputs["Wo"], dtype=np.float32),
        "bo": np.ascontiguousarray(inputs["bo"], dtype=np.float32),
        "gamma": np.ascontiguousarray(inputs["gamma"], dtype=np.float32),
        "beta": np.ascontiguousarray(inputs["beta"], dtype=np.float32),
    }
    in_maps = []
    for i in range(NCORES):
        m = dict(shared)
        m["xb"] = np.ascontiguousarray(x[i * BPC:(i + 1) * BPC])
        m["w2s"] = np.ascontiguousarray(w2[:, i * W2C:(i + 1) * W2C])
        m["b2s"] = np.ascontiguousarray(b2[i * W2C:(i + 1) * W2C])
        in_maps.append(m)
    return in_maps


def _run(inputs, **kwargs):
    nc = _get_nc()
    res = run_bass_kernel_spmd(nc, _shard(inputs), core_ids=list(range(NCORES)),
                               **kwargs)
    outs = [r["out"] for r in res.results]
    full = np.concatenate(outs, axis=0).reshape(NCORES * BPC, T, C)
    return full, res


def kernel(**inputs) -> np.ndarray:
    out, _ = _run(inputs)
    return out


# revision 17
# speedup vs baseline: 1.0726x; 1.0726x over previous
"""Trainium2 Bass kernel for nn_LowRankDynamicConv.

Math (per sample b):
  combined = [phrase_slot[b] | eos]                       [N, 2C]
  h        = relu(combined @ W1 + b1)                     [N, 4C]
  proj     = (h @ W2 + b2) viewed as [N*C, R]             [4096, 32]
  y        = x[b] @ proj   with x[b] = context_emb[b] as  [T, N*C]
  out_k[t] = sum_j y[t + j - pad_k] @ kparam_k[:, :, j]   [T, C] for k in (1,3,5)
  out      = relu(LN(concat(out_k) @ Wo + bo))            [T, C]

This is the low-rank refactor of the reference's dense dynamic conv:
  out_k = sum_j shift_j(x) @ (proj @ kparam_k[:,:,j]) == sum_j shift_j(x @ proj) @ kparam_k

Sharding: data-parallel over batch, 2 samples per core. W2 (the one big
weight, 32MB) is column-sharded: every core computes the proj slice for
ALL 16 samples with its 1/8 of W2's columns, then an on-chip AllToAll
redistributes so each core holds the full proj for its own 2 samples.
This cuts per-core HBM traffic by ~28MB vs replicating W2.

Precision: weights and activations feeding matmuls are cast to bf16
(fast weight load + 1 cyc/row); accumulation stays f32 in PSUM, and the
LN epilogue is f32. x is PE-transposed in f32 and stored bf16. Measured
end-to-end rel err vs the f32 reference is well under the 2e-2 gate.
"""
import sys

sys.path.insert(0, "/opt/trn_rl_repo")

import numpy as np

import concourse.bass as bass
import concourse.mybir as mybir
import concourse.tile as tile
from concourse import bacc
from concourse.bass_utils import run_bass_kernel_spmd
from concourse.masks import make_identity

F32 = mybir.dt.float32
BF16 = mybir.dt.bfloat16
RELU = mybir.ActivationFunctionType.Relu
SQRT = mybir.ActivationFunctionType.Sqrt

NCORES = 8
BPC = 2                    # samples per core
T, N, C, R = 1024, 16, 256, 32
BN = NCORES * BPC * N      # 256 (b, n) rows across ALL samples
NCF = N * C                # 4096 flattened (n, c) contraction dim
CH = NCF // 128            # 32 nc-chunks of 128
TQ = 2                     # t processed in 2 chunks of 512
TCHUNK = T // TQ           # 512
PAD = 2                    # max conv pad (k=5)
YW = T + 2 * PAD           # padded y width, 1028
W2C = C * R // NCORES      # 1024 W2 columns per core
# (kernel_size, j) pairs in feat-concat order: k1 | k3 | k5
KJ = [(1, [0]), (3, [0, 1, 2]), (5, [0, 1, 2, 3, 4])]
NJ = 9                     # total j count


def _broadcast_ap(ap, parts):
    """DMA access pattern replicating a 1D/2D DRAM tensor across `parts` partitions."""
    a = ap
    return bass.AP(tensor=a.tensor, offset=a.offset, ap=[[0, parts]] + list(a.ap))


def _build():
    nc = bacc.Bacc("TRN2", num_devices=NCORES)

    xb = nc.dram_tensor("xb", [BPC, T, NCF], F32, kind="ExternalInput")
    phrase = nc.dram_tensor("phrase", [BN, C], F32, kind="ExternalInput")
    eos = nc.dram_tensor("eos", [C], F32, kind="ExternalInput")
    w1 = nc.dram_tensor("w1", [2 * C, 4 * C], F32, kind="ExternalInput")
    b1 = nc.dram_tensor("b1", [4 * C], F32, kind="ExternalInput")
    w2s = nc.dram_tensor("w2s", [4 * C, W2C], F32, kind="ExternalInput")
    b2s = nc.dram_tensor("b2s", [W2C], F32, kind="ExternalInput")
    kjoin = nc.dram_tensor("kjoin", [NJ, R, C], F32, kind="ExternalInput")
    wo = nc.dram_tensor("wo", [3 * C, C], F32, kind="ExternalInput")
    bo = nc.dram_tensor("bo", [C], F32, kind="ExternalInput")
    gamma = nc.dram_tensor("gamma", [C], F32, kind="ExternalInput")
    beta = nc.dram_tensor("beta", [C], F32, kind="ExternalInput")
    out = nc.dram_tensor("out", [BPC, T, C], F32, kind="ExternalOutput")

    with tile.TileContext(nc) as tc:
        with tc.tile_pool(name="keep", bufs=1) as keep, \
             tc.tile_pool(name="dram", bufs=1, space="DRAM") as dram, \
             tc.tile_pool(name="tp", bufs=2, space="PSUM") as tp, \
             tc.tile_pool(name="pXn", bufs=2) as pXn, \
             tc.tile_pool(name="pX", bufs=2) as pX:
            ident = keep.tile([128, 128], F32)
            make_identity(nc, ident)
            ones1 = keep.tile([1, 128], BF16)
            nc.vector.memset(ones1, 1.0)
            eps = keep.tile([128, 1], F32)
            nc.vector.memset(eps, 1e-5)

            # LN params + output bias, broadcast across partitions
            gsb = keep.tile([128, C], F32)
            nc.sync.dma_start(gsb, _broadcast_ap(gamma[:], 128))
            bsb = keep.tile([128, C], F32)
            nc.sync.dma_start(bsb, _broadcast_ap(beta[:], 128))
            bosb = keep.tile([128, C], F32)
            nc.sync.dma_start(bosb, _broadcast_ap(bo[:], 128))

            # bf16 stage-4/5 weights (loaded in phase A via the shared scratch tag)
            wob = keep.tile([128, 6, C], BF16)
            kjb = keep.tile([R, NJ, C], BF16)

            # y^T buffers, one per sample: [r=32 part, padded t] in bf16
            ysb = []
            for b in range(BPC):
                y = keep.tile([R, YW], BF16, name=f"ysb{b}")
                nc.vector.memset(y[:, 0:PAD], 0.0)
                nc.vector.memset(y[:, YW - PAD:YW], 0.0)
                ysb.append(y)

            # proj for this core's samples: [nc%128 part, (b, ch), r], bf16
            projw = keep.tile([128, BPC * CH, R], BF16)

            # AllToAll bounce buffers: [src/dst core, bn rows, W2-col slice]
            in_b = dram.tile([NCORES, BPC * N, W2C], BF16, tag="inb")
            out_b = dram.tile([NCORES, BPC * N, W2C], BF16, tag="outb")

            # ---- phase A: proj slices for all samples, AllToAll, reshard ----
            with tc.tile_pool(name="pA", bufs=1) as pA, \
                 tc.tile_pool(name="pAs", bufs=2) as pAs, \
                 tc.tile_pool(name="psA", bufs=2, space="PSUM") as psA, \
                 tc.high_priority():
                # combined^T [c2%128 part, ko, bn] for ALL bn rows, bf16
                phsb = pA.tile([128, 2, C], F32)
                nc.sync.dma_start(phsb, phrase[:, :].rearrange("(rt p) c -> p rt c", p=128))
                eossb = pA.tile([128, 2], F32)
                nc.sync.dma_start(eossb, eos[:].rearrange("(o p) -> p o", p=128))
                combT = pA.tile([128, 4, BN], BF16)
                for rt in range(2):
                    for ko in range(2):
                        pht = psA.tile([128, 128], F32, tag="h")
                        nc.tensor.transpose(pht, phsb[:, rt, ko * 128:(ko + 1) * 128],
                                            ident)
                        nc.vector.tensor_copy(combT[:, ko, rt * 128:(rt + 1) * 128], pht)
                for o in range(2):
                    nc.vector.tensor_copy(
                        combT[:, 2 + o, :],
                        eossb[:, o:o + 1].to_broadcast((128, BN)))

                # W1 -> bf16 [c2%128, ko, m]; b1 -> [m%128, mo]
                w1b = pA.tile([128, 4, 4 * C], BF16)
                for hw in range(2):
                    w1q = keep.tile([128, 2, 4 * C], F32, tag="stg", name=f"w1q{hw}")
                    nc.sync.dma_start(
                        w1q, w1[hw * 256:(hw + 1) * 256, :]
                        .rearrange("(ko p) m -> p ko m", p=128))
                    nc.vector.tensor_copy(w1b[:, 2 * hw:2 * hw + 2, :], w1q)
                b1sb = pA.tile([128, 8], F32)
                nc.sync.dma_start(b1sb, b1[:].rearrange("(mo p) -> p mo", p=128))

                # h^T [m%128 part, mo, bn] = relu(W1^T combined + b1), bf16
                hT = pA.tile([128, 8, BN], BF16)
                for mo in range(8):
                    phm = psA.tile([128, BN], F32, tag="h")
                    for ko in range(4):
                        nc.tensor.matmul(phm, w1b[:, ko, mo * 128:(mo + 1) * 128],
                                         combT[:, ko, :],
                                         start=(ko == 0), stop=(ko == 3))
                    nc.scalar.activation(out=hT[:, mo, :], in_=phm, func=RELU,
                                         bias=b1sb[:, mo:mo + 1], scale=1.0)

                # b2 -> bf16 (staged early; the rank-1 bias add closes each group)
                b2f = keep.tile([1, W2C], F32, tag="stg", name="b2f")
                nc.sync.dma_start(b2f, b2s[:].rearrange("(p q) -> p q", p=1))
                b2b = pA.tile([1, W2C], BF16)
                nc.vector.tensor_copy(b2b, b2f)

                # W2 slice -> bf16 in quarters; pp accumulates as quarters land
                w2b = pA.tile([128, 8, W2C], BF16)
                groups = [(0, 0), (0, 1), (1, 0), (1, 1)]
                pps = [psA.tile([128, 512], F32, tag="pj", bufs=4, name=f"pp{g}")
                       for g in range(4)]
                for qw in range(4):
                    w2q = pA.tile([128, 2, W2C], F32, tag="w2q", bufs=2, name=f"w2q{qw}")
                    nc.sync.dma_start(
                        w2q, w2s[qw * 256:(qw + 1) * 256, :]
                        .rearrange("(ko p) q -> p ko q", p=128))
                    nc.vector.tensor_copy(w2b[:, 2 * qw:2 * qw + 2, :], w2q)
                    for g, (rt, cc) in enumerate(groups):
                        for k2 in range(2):
                            ko = 2 * qw + k2
                            nc.tensor.matmul(
                                pps[g], hT[:, ko, rt * 128:(rt + 1) * 128],
                                w2b[:, ko, cc * 512:(cc + 1) * 512],
                                start=(ko == 0), stop=False)
                for g, (rt, cc) in enumerate(groups):
                    nc.tensor.matmul(pps[g], ones1[:, 0:128],
                                     b2b[:, cc * 512:(cc + 1) * 512],
                                     start=False, stop=True)
                    ppsb = pAs.tile([128, 512], BF16, tag="ppsb")
                    nc.scalar.copy(ppsb, pps[g])
                    nc.scalar.dma_start(
                        in_b[rt * 4:(rt + 1) * 4, :, cc * 512:(cc + 1) * 512], ppsb)
                wof = keep.tile([128, 6, C], F32, tag="stg", name="wof")
                nc.sync.dma_start(wof, wo[:, :].rearrange("(fc p) co -> p fc co", p=128))
                nc.vector.tensor_copy(wob, wof)
                kjf = keep.tile([R, NJ, C], F32, tag="stg", name="kjf")
                nc.sync.dma_start(kjf, kjoin[:, :, :].rearrange("j r d -> r j d"))
                nc.vector.tensor_copy(kjb, kjf)

                nc.gpsimd.collective_compute(
                    "AllToAll",
                    mybir.AluOpType.bypass,
                    replica_groups=[list(range(NCORES))],
                    ins=[in_b[:, :, :].opt()],
                    outs=[out_b[:, :, :].opt()],
                )

                # reshard: out_b[i=qh, b*16+n, (2*ql+e)*32+r] -> projw[qh*16+ql, (b,n,e), r]
                # i.e. partition q holds c = 2q+e; pairs match the bf16-pair transposes.
                ob = out_b[:, :, :]
                for qh in range(NCORES):
                    src = bass.AP(
                        tensor=ob.tensor,
                        offset=ob.offset + qh * (BPC * N * W2C),
                        ap=[[64, 16],            # ql -> partition within the qh-group
                            [W2C, BPC * N],      # (b, n) merged
                            [1, 2 * R]])         # (e, r) merged
                    nc.scalar.dma_start(projw[qh * 16:(qh + 1) * 16, :, :], src)

            # ---- phase X: per (sample, t-chunk) stream ------------------------
            with tc.tile_pool(name="yp", bufs=2, space="PSUM") as yp, \
                 tc.tile_pool(name="fp", bufs=2, space="PSUM") as fp, \
                 tc.tile_pool(name="op", bufs=2, space="PSUM") as op, \
                 tc.tile_pool(name="pX2", bufs=1) as pX2, \
                 tc.tile_pool(name="pXs", bufs=2) as pXs:
                def emit_feat(b, tq):
                    t0 = tq * TCHUNK
                    # stage 4: feat^T[d-block, t] = sum_j kjoin_j^T @ shift(y^T)
                    featT = pX.tile([128, 6, TCHUNK], BF16, tag="featT", bufs=1)
                    jj = 0
                    for kb, (k, js) in enumerate(KJ):
                        pad = k // 2
                        for dc in range(2):
                            pf = fp.tile([128, TCHUNK], F32, tag="f")
                            for ji, j in enumerate(js):
                                ys = ysb[b][:, PAD + t0 + j - pad:
                                            PAD + t0 + j - pad + TCHUNK]
                                nc.tensor.matmul(
                                    pf, kjb[:, jj + ji, dc * 128:(dc + 1) * 128],
                                    ys, start=(ji == 0), stop=(ji == len(js) - 1))
                            nc.vector.tensor_copy(featT[:, kb * 2 + dc, :], pf)
                        jj += len(js)

                    # stage 5: out[t%128, co] = feat @ Wo, then LN + relu
                    for ts in range(TCHUNK // 128):
                        po = op.tile([128, C], F32, tag="o")
                        for fc in range(6):
                            nc.tensor.matmul(
                                po, featT[:, fc, ts * 128:(ts + 1) * 128],
                                wob[:, fc, :], start=(fc == 0), stop=(fc == 5))
                        osb = pXs.tile([128, C], F32, tag="osb")
                        nc.vector.tensor_add(osb, po, bosb)
                        st = pXs.tile([128, 6], F32, tag="st")
                        nc.vector.bn_stats(out=st, in_=osb)
                        mv = pXs.tile([128, 2], F32, tag="mv")
                        nc.vector.bn_aggr(out=mv, in_=st)
                        # rstd = 1/sqrt(var + eps)
                        rs = pXs.tile([128, 1], F32, tag="rs")
                        nc.scalar.activation(out=rs, in_=mv[:, 1:2], func=SQRT,
                                             bias=eps, scale=1.0)
                        nc.vector.reciprocal(rs, rs)
                        nc.vector.tensor_scalar(osb, osb, mv[:, 0:1], rs,
                                                mybir.AluOpType.subtract,
                                                mybir.AluOpType.mult)
                        nc.vector.tensor_mul(osb, osb, gsb)
                        nc.vector.tensor_add(osb, osb, bsb)
                        nc.vector.tensor_scalar_max(osb, osb, 0.0)
                        nc.sync.dma_start(
                            out[b, t0 + ts * 128:t0 + (ts + 1) * 128, :], osb)

                for b in range(BPC):
                    for tq in range(TQ):
                        chunk = b * TQ + tq
                        t0 = tq * TCHUNK
                        # x [t%128, nc] f32 -> bf16, then PE-transpose bf16 PAIRS
                        # viewed as f32 (bit-exact): xT2[q, n, t] = packed
                        # (x[t, n*256+2q], x[t, n*256+2q+1])
                        xpool = pX if chunk < 2 else pX2
                        xT2 = xpool.tile([128, N, TCHUNK], F32, tag="xT")
                        for ts in range(TCHUNK // 128):
                            xn = pXn.tile([128, NCF], F32, tag="xn")
                            nc.sync.dma_start(
                                xn, xb[b, t0 + ts * 128:t0 + (ts + 1) * 128, :])
                            xc = pXn.tile([128, NCF], BF16, tag="xc")
                            # early chunks: cast on ACT (DVE feeds phase A's
                            # w2->bf16); later chunks: cast on DVE
                            if chunk < 2:
                                nc.scalar.copy(xc, xn)
                            else:
                                nc.vector.tensor_copy(xc, xn)
                            xc32 = xc[:, :].bitcast(F32)
                            for pg in range(4):   # 4 pair-block transposes per bank
                                pt = tp.tile([128, 4, 128], F32, tag="tp")
                                for q in range(4):
                                    blk = pg * 4 + q
                                    nc.tensor.transpose(
                                        pt[:, q, :],
                                        xc32[:, blk * 128:(blk + 1) * 128], ident)
                                dst = xT2[:, pg * 4:(pg + 1) * 4,
                                          ts * 128:(ts + 1) * 128]
                                if chunk < 2 or pg % 2 == 0:
                                    nc.vector.tensor_copy(dst, pt)
                                else:
                                    nc.scalar.copy(dst, pt)

                        # stage 3: y^T[r, t-chunk] = sum_(n,e) proj^T @ xT2
                        # (even/odd bf16 lanes of each packed pair via stride-2 APs)
                        xv = xT2[:, :, :].bitcast(BF16)  # [128, N, 2*TCHUNK]
                        py = yp.tile([R, TCHUNK], F32, tag="y")
                        for pc in range(N):
                            for e in range(2):
                                rhs = xv[:, pc, :].rearrange(
                                    "p (t e) -> p e t", e=2)[:, e, :]
                                nc.tensor.matmul(
                                    py, projw[:, (b * N + pc) * 2 + e, :], rhs,
                                    start=(pc == 0 and e == 0),
                                    stop=(pc == N - 1 and e == 1))
                        nc.vector.tensor_copy(ysb[b][:, PAD + t0:PAD + t0 + TCHUNK],
                                              py)
                        # stage 4/5 lag one chunk: feat(tq-1) needs y[tq]'s
                        # first PAD columns (k=5 right overhang)
                        if tq > 0:
                            emit_feat(b, tq - 1)
                    emit_feat(b, TQ - 1)

    nc.compile()
    return nc


_NC = None


def _get_nc():
    global _NC
    if _NC is None:
        _NC = _build()
    return _NC


def _shard(inputs):
    """Split full inputs into per-core input maps (pure slicing/stacking)."""
    x = np.ascontiguousarray(inputs["context_emb"], dtype=np.float32)
    B = x.shape[0]
    assert B == NCORES * BPC
    x = x.reshape(B, T, NCF)
    ph = np.ascontiguousarray(
        inputs["phrase_slot"], dtype=np.float32).reshape(BN, C)
    w2 = np.asarray(inputs["W2"], dtype=np.float32)
    b2 = np.asarray(inputs["b2"], dtype=np.float32)
    kjoin = np.ascontiguousarray(np.concatenate(
        [np.moveaxis(inputs[f"k{k}"], 2, 0) for k in (1, 3, 5)], axis=0),
        dtype=np.float32)  # [9, 32, 256]
    shared = {
        "phrase": ph,
        "eos": np.ascontiguousarray(inputs["eos_slot"].reshape(C), dtype=np.float32),
        "w1": np.ascontiguousarray(inputs["W1"], dtype=np.float32),
        "b1": np.ascontiguousarray(inputs["b1"], dtype=np.float32),
        "kjoin": kjoin,
        "wo": np.ascontiguousarray(inputs["Wo"], dtype=np.float32),
        "bo": np.ascontiguousarray(inputs["bo"], dtype=np.float32),
        "gamma": np.ascontiguousarray(inputs["gamma"], dtype=np.float32),
        "beta": np.ascontiguousarray(inputs["beta"], dtype=np.float32),
    }
    in_maps = []
    for i in range(NCORES):
        m = dict(shared)
        m["xb"] = np.ascontiguousarray(x[i * BPC:(i + 1) * BPC])
        m["w2s"] = np.ascontiguousarray(w2[:, i * W2C:(i + 1) * W2C])
        m["b2s"] = np.ascontiguousarray(b2[i * W2C:(i + 1) * W2C])
        in_maps.append(m)
    return in_maps


def _run(inputs, **kwargs):
    nc = _get_nc()
    res = run_bass_kernel_spmd(nc, _shard(inputs), core_ids=list(range(NCORES)),
                               **kwargs)
    outs = [r["out"] for r in res.results]
    full = np.concatenate(outs, axis=0).reshape(NCORES * BPC, T, C)
    return full, res


def kernel(**inputs) -> np.ndarray:
    out, _ = _run(inputs)
    return out
